# revision 6
# baseline (speedup 1.0000x reference)
"""Masked self-attention Trainium2 kernel (v6 — lane-balanced split-K).

Reference computes (per batch b):
    key   = x @ Wk.T            [S, 64]
    query = x @ Wq.T            [S, 64]
    value = x @ Wv.T            [S, 128]
    kT_m  = tril(key.T)         [64, S]   -- element (d, s) kept iff s <= d
    out   = softmax(query @ kT_m, axis=-1) @ value

tril zeroes every score column s >= 64, so with fixed shift c:

    out[t] = (sum_{s<64} e^{z_st-c} v[s] + e^{-c} Vtail) /
             (sum_{s<64} e^{z_st-c}      + e^{-c} (S-64))

with Vtail = (sum_{s>=64} x[s]) @ Wv.T.  Per core (batch b, half h).

v6 design (trace/cost-model driven):
- Input split so each consumer chain unblocks earliest: wpkA (x64T|WkT|Wq|tri,
  80 KiB) first on the sync HWDGE queue -> score-weight preamble ready ~2.5us;
  xow (own half transposed, fp16) alone on the scalar HWDGE queue -> z chunks;
  other half as fp8 natural-TILED blocks (token-major) split across the
  gpsimd SWDGE queue and sync queue.
- Other-half column sum on the PE: 16 rank-reduce matmuls (lhsT = 128-token
  fp8 block, rhs = fp8 ones column) accumulating one PSUM [128,1] -- frees
  the scalar engine to run the 4 exps back-to-back right after the z matmuls.
- Own-half sums: DVE reduce (first 1024) + Pool reduce (second 1024).
- Split-K output tiles: 12 early tiles run K=64 (value rows only) as exps
  land, then one rank-1 close per tile (lhsT = pT's constant e^{-c} row 64,
  rhs = [vtail|NTAIL]) once the tail lands; the 4 last tiles (gated on exp3
  anyway) run K=65 directly.
- 3 tiles per PSUM bank [128, 387] -> one strided [128,3] reciprocal per
  bank; scales split across DVE and ACT; stores spread over all 3 queues.
- Long PE warmup chain (matmuls on a memset tile) keeps the tensor engine
  ramping toward full clock before the real work arrives.
"""

import numpy as np

import concourse.bass as bass
import concourse.bacc as bacc
import concourse.tile as tile
from concourse import mybir
from concourse.bass_utils import run_bass_kernel_spmd

F32 = mybir.dt.float32
F16 = mybir.dt.float16
BF16 = mybir.dt.bfloat16
FP8 = mybir.dt.float8e4
AF = mybir.ActivationFunctionType
AX = mybir.AxisListType
ALU = mybir.AluOpType

B, S, E, KD = 4, 4096, 128, 64
HALF = S // 2            # tokens handled per core
NCORES = 8
CHUNK = 512              # tokens per z-matmul / exp
NCHUNK = HALF // CHUNK
TSUB = 128               # tokens per output tile
NTILE = HALF // TSUB     # 16
CSHIFT = 20.0            # fixed softmax shift
NTAIL = float(S - KD)    # 4032 all-zero score columns
W = E + 1                # 129: num cols + den col per tile
NBANK = 6                # 3 tiles per PSUM bank (last bank holds 1)

# wpkA columns: [x64T(64) | WkT(64) | Wq(128, rows 0:64) | tri(64, rows 0:64)]
X64_OFF, WK_OFF, WQ_OFF, TRI_OFF = 0, KD, 2 * KD, 2 * KD + E
WPKA_COLS = 2 * KD + E + KD  # 320

NWARM = 9                # PE warmup matmuls (ramp toward full clock)


def _build_nc() -> bass.Bass:
    nc = bacc.Bacc("TRN2", target_bir_lowering=False, debug=False)

    wpka = nc.dram_tensor("wpka", [E, WPKA_COLS], F16, kind="ExternalInput").ap()
    wpkb = nc.dram_tensor("wpkb", [E, E], F16, kind="ExternalInput").ap()
    xow = nc.dram_tensor("xow", [E, HALF], F16, kind="ExternalInput").ap()
    xo8a = nc.dram_tensor("xo8a", [E, HALF // 2], FP8, kind="ExternalInput").ap()
    xo8b = nc.dram_tensor("xo8b", [E, HALF // 2], FP8, kind="ExternalInput").ap()
    outs = [
        nc.dram_tensor(f"o{g}", [TSUB, 4, E], BF16, kind="ExternalOutput").ap()
        for g in range(4)
    ]

    with tile.TileContext(nc) as tc:
        with (
            tc.tile_pool(name="singles", bufs=1) as singles,
            tc.tile_pool(name="zps", bufs=2, space="PSUM") as zps,
            tc.tile_pool(name="misc_ps", bufs=1, space="PSUM") as misc_ps,
            tc.tile_pool(name="oa_ps", bufs=4, space="PSUM") as oa_ps,
            tc.tile_pool(name="recs", bufs=3) as recs,
            tc.tile_pool(name="obs", bufs=3) as obs,
        ):
            # ---- DMA in (queue order == issue order per engine) ----
            wpka_sb = singles.tile([E, WPKA_COLS], F16)
            nc.sync.dma_start(wpka_sb[:], wpka)
            wpkb_sb = singles.tile([E, E], F16)
            nc.sync.dma_start(wpkb_sb[:], wpkb)
            xo8b_sb = singles.tile([E, HALF // 2], FP8)
            nc.sync.dma_start(xo8b_sb[:], xo8b)
            xow_sb = singles.tile([E, HALF], F16)
            nc.scalar.dma_start(xow_sb[:], xow)
            xo8a_sb = singles.tile([E, HALF // 2], FP8)
            nc.gpsimd.dma_start(xo8a_sb[:], xo8a)

            x64T_sb = wpka_sb[:, X64_OFF : X64_OFF + KD]
            wkT_sb = wpka_sb[:, WK_OFF : WK_OFF + KD]
            wq_sb = wpka_sb[0:KD, WQ_OFF : WQ_OFF + E]
            tri_sb = wpka_sb[0:KD, TRI_OFF : TRI_OFF + KD]

            # ---- constants (gpsimd is otherwise idle early) ----
            wfil_sb = singles.tile([E, 448], F16)
            nc.gpsimd.memset(wfil_sb[:], 0.001)
            wzaug_sb = singles.tile([E, KD + 1], F16)
            nc.gpsimd.memset(wzaug_sb[:, KD : KD + 1], 0.0)
            vaug_sb = singles.tile([KD + 1, W], BF16)
            nc.gpsimd.memset(vaug_sb[0:KD, E : E + 1], 1.0)
            nc.gpsimd.memset(vaug_sb[KD : KD + 1, E : E + 1], NTAIL)
            nbias_sb = singles.tile([KD + 1, 1], F32)
            nc.gpsimd.memset(nbias_sb[:], -CSHIFT)
            ones8_sb = singles.tile([E, 1], FP8)
            nc.gpsimd.memset(ones8_sb[:], 1.0)

            # ---- PE warmup chain: matmuls on the memset filler tile keep
            # the tensor engine continuously busy from t~0.3us so it reaches
            # a high p-state before the first real matmul.
            def warm(i):
                wp = zps.tile([E, 448], F32, tag="z", name=f"warm{i}_ps")
                nc.tensor.matmul(
                    wp[:], wfil_sb[:, 0:E], wfil_sb[:], start=True, stop=True
                )

            for i in range(5):
                warm(i)

            # ---- preamble: build Wz (score weights) and v64 ----
            kT_ps = zps.tile([KD, KD], F32, tag="z", name="kT_ps")
            nc.tensor.matmul(kT_ps[:], wkT_sb, x64T_sb, start=True, stop=True)
            kmT_sb = singles.tile([KD, KD], F16)
            nc.vector.tensor_mul(kmT_sb[:], kT_ps[:], tri_sb)
            warm(5)
            wzT_ps = zps.tile([E, KD], F32, tag="z", name="wzT_ps")
            nc.tensor.matmul(wzT_ps[:], wq_sb, kmT_sb[:], start=True, stop=True)
            nc.vector.tensor_copy(wzaug_sb[:, 0:KD], wzT_ps[:])
            warm(6)
            v64_ps = zps.tile([KD, E], F32, tag="z", name="v64_ps")
            nc.tensor.matmul(v64_ps[:], x64T_sb, wpkb_sb[:], start=True, stop=True)
            nc.scalar.activation(vaug_sb[0:KD, 0:E], v64_ps[:], AF.Copy)
            warm(7)
            warm(8)

            x64s_sb = singles.tile([E, 1], F32)
            nc.vector.reduce_sum(out=x64s_sb[:], in_=x64T_sb, axis=AX.X)

            # ---- z chunks + exps (z banks ping-pong; z2 reuses z0's bank
            # once exp0 has consumed it) ----
            pT_sb = singles.tile([KD + 1, HALF], BF16)
            z_tiles = []
            for c in range(NCHUNK):
                cs = slice(c * CHUNK, (c + 1) * CHUNK)
                z_ps = zps.tile([KD + 1, CHUNK], F32, tag="z", name=f"z{c}_ps")
                nc.tensor.matmul(
                    z_ps[:], wzaug_sb[:], xow_sb[:, cs], start=True, stop=True
                )
                nc.scalar.activation(
                    pT_sb[0 : KD + 1, cs], z_ps[:], AF.Exp, bias=nbias_sb[:]
                )
                z_tiles.append(z_ps)

                if c == 1:
                    # other-half sums on the PE while z2/z3 wait for data:
                    # 16 fp8 block x ones matmuls accumulate one [128,1].
                    osum_ps = misc_ps.tile([E, 1], F32, name="osum_ps")
                    for j in range(8):
                        js = slice(j * TSUB, (j + 1) * TSUB)
                        nc.tensor.matmul(
                            osum_ps[:], xo8a_sb[:, js], ones8_sb[:],
                            start=(j == 0), stop=False,
                        )
                    for j in range(8):
                        js = slice(j * TSUB, (j + 1) * TSUB)
                        nc.tensor.matmul(
                            osum_ps[:], xo8b_sb[:, js], ones8_sb[:],
                            start=False, stop=(j == 7),
                        )

            # ---- own-half sums (DVE + Pool) and the tail vector ----
            rdA_sb = singles.tile([E, 1], F32)
            nc.vector.reduce_sum(out=rdA_sb[:], in_=xow_sb[:, 0:1024], axis=AX.X)
            rdB_sb = singles.tile([E, 1], F32)
            nc.vector.reduce_sum(out=rdB_sb[:], in_=xow_sb[:, 1024:2048], axis=AX.X)
            u1_sb = singles.tile([E, 1], F32)
            nc.vector.scalar_tensor_tensor(
                u1_sb[:], rdA_sb[:], rdB_sb[:], osum_ps[:], ALU.add, ALU.add
            )
            tailh_sb = singles.tile([E, 1], F16)
            nc.vector.tensor_sub(tailh_sb[:], u1_sb[:], x64s_sb[:])

            # ---- output tiles: 3 tiles per PSUM bank [128, 3*129] ----
            oa_banks = []
            rec_tiles = []
            ob_tiles = []
            scale_engs = []

            def bank_of(t):
                return oa_banks[t // 3]

            def slot_ap(t):
                bank = bank_of(t)
                j = t % 3
                return bank[:, j * W : j * W + W]

            def emit_tile(t, k65):
                if t % 3 == 0:
                    oa_banks.append(
                        oa_ps.tile([TSUB, 3 * W], F32, tag="oa", name=f"oa{t // 3}")
                    )
                ts = slice(t * TSUB, (t + 1) * TSUB)
                oa = slot_ap(t)
                if k65:
                    nc.tensor.matmul(
                        oa, pT_sb[0 : KD + 1, ts], vaug_sb[:], start=True, stop=True
                    )
                else:
                    nc.tensor.matmul(
                        oa, pT_sb[0:KD, ts], vaug_sb[0:KD, :], start=True, stop=False
                    )

            def emit_close(t):
                ts = slice(t * TSUB, (t + 1) * TSUB)
                nc.tensor.matmul(
                    slot_ap(t), pT_sb[KD : KD + 1, ts], vaug_sb[KD : KD + 1, :],
                    start=False, stop=True,
                )

            # tail vector: vtail = tailh @ Wv.T, written into vaug row 64
            vtail_ps = misc_ps.tile([1, E], F32, name="vtail_ps")
            nc.tensor.matmul(vtail_ps[:], tailh_sb[:], wpkb_sb[:], start=True, stop=True)
            nc.vector.tensor_copy(vaug_sb[KD : KD + 1, 0:E], vtail_ps[:])

            for t in range(NTILE):
                emit_tile(t, k65=True)

            # ---- normalize + store ----
            # per-bank strided reciprocal of the 3 den cols
            for b in range(NBANK):
                n = 3 if b < 5 else 1
                rec = recs.tile([TSUB, 3], F32, tag="rec", name=f"rec{b}")
                rec_tiles.append(rec)
                bank = oa_banks[b]
                nc.vector.reciprocal(
                    rec[:, 0:n], bank[:, E :: W][:, 0:n]
                )

            for t in range(NTILE):
                g, gj = divmod(t, 4)
                if gj == 0:
                    ob_tiles.append(
                        obs.tile([TSUB, 4, E], BF16, tag="ob", name=f"ob{g}")
                    )
                ob = ob_tiles[g]
                oa = slot_ap(t)
                rec = rec_tiles[t // 3][:, t % 3 : t % 3 + 1]
                if t % 2 == 0:
                    nc.vector.tensor_scalar_mul(ob[:, gj, :], oa[:, 0:E], rec)
                else:
                    nc.scalar.activation(ob[:, gj, :], oa[:, 0:E], AF.Copy, scale=rec)
                if gj == 3:
                    eng = (nc.scalar, nc.sync, nc.gpsimd, nc.scalar)[g]
                    eng.dma_start(outs[g], ob[:])

    nc.compile()
    return nc


_NC_CACHE = None


def _get_nc() -> bass.Bass:
    global _NC_CACHE
    if _NC_CACHE is None:
        _NC_CACHE = _build_nc()
    return _NC_CACHE


def _make_in_maps(x, Wk, Wq, Wv):
    tri = (np.arange(KD)[:, None] >= np.arange(KD)[None, :]).astype(np.float16)
    wq_pad = np.zeros((E, E), np.float16)
    wq_pad[:KD] = Wq.astype(np.float16)
    tri_pad = np.zeros((E, KD), np.float16)
    tri_pad[:KD] = tri
    x16 = x.astype(np.float16)
    fp8_np = mybir.dt.np(FP8)
    wvT = np.ascontiguousarray(Wv.T.astype(np.float16))
    in_maps = []
    for c in range(NCORES):
        b, h = divmod(c, 2)
        xb_ = x16[b]
        wpka = np.concatenate(
            [xb_[:KD].T, Wk.T.astype(np.float16), wq_pad, tri_pad], axis=1
        )
        own = xb_[h * HALF : (h + 1) * HALF].T  # [E, 2048]
        other = xb_[(1 - h) * HALF : (2 - h) * HALF]  # [2048, E] natural
        # natural-tiled fp8: block j holds tokens j*128..j*128+127 on the
        # partition axis: pack[p, j*128+e] = other[j*128+p, e]
        ot = other.astype(fp8_np).reshape(16, TSUB, E).transpose(1, 0, 2)
        ot = np.ascontiguousarray(ot).reshape(E, HALF)
        in_maps.append(
            {
                "wpka": np.ascontiguousarray(wpka),
                "wpkb": wvT,
                "xow": np.ascontiguousarray(own),
                "xo8a": np.ascontiguousarray(ot[:, 0 : HALF // 2]),
                "xo8b": np.ascontiguousarray(ot[:, HALF // 2 : HALF]),
            }
        )
    return in_maps


def _gather(results):
    out = np.empty((B, S, E), np.float32)
    for c, r in enumerate(results):
        b, h = divmod(c, 2)
        # per-group device layout [p, t, v], token = (4g + t)*128 + p
        dev = np.concatenate(
            [np.asarray(r[f"o{g}"], dtype=np.float32) for g in range(4)], axis=1
        )
        out[b, h * HALF : (h + 1) * HALF] = dev.transpose(1, 0, 2).reshape(HALF, E)
    return out


def _run(x, Wk, Wq, Wv, **spmd_kwargs):
    nc = _get_nc()
    res = run_bass_kernel_spmd(
        nc,
        _make_in_maps(x, Wk, Wq, Wv),
        core_ids=list(range(NCORES)),
        **spmd_kwargs,
    )
    return _gather(res.results), res


def kernel(x, Wk, Wq, Wv):
    x = np.ascontiguousarray(np.asarray(x), dtype=np.float32)
    Wk = np.ascontiguousarray(np.asarray(Wk), dtype=np.float32)
    Wq = np.ascontiguousarray(np.asarray(Wq), dtype=np.float32)
    Wv = np.ascontiguousarray(np.asarray(Wv), dtype=np.float32)
    out, _ = _run(x, Wk, Wq, Wv)
    return out


# revision 7
# speedup vs baseline: 1.1510x; 1.1510x over previous
"""Masked self-attention Trainium2 kernel (v6.1 — balanced queues, K=65 tiles).

Reference computes (per batch b):
    key   = x @ Wk.T            [S, 64]
    query = x @ Wq.T            [S, 64]
    value = x @ Wv.T            [S, 128]
    kT_m  = tril(key.T)         [64, S]   -- element (d, s) kept iff s <= d
    out   = softmax(query @ kT_m, axis=-1) @ value

tril zeroes every score column s >= 64, so with fixed shift c:

    out[t] = (sum_{s<64} e^{z_st-c} v[s] + e^{-c} Vtail) /
             (sum_{s<64} e^{z_st-c}      + e^{-c} (S-64))

with Vtail = (sum_{s>=64} x[s]) @ Wv.T.  Per core (batch b, half h).

Trace-driven v6.1 notes:
- Queues (~150 GB/s HWDGE each, ~100 GB/s gpsimd SWDGE; ~0.9us DMA-sem
  latency each): sync: wpk -> xo8b -> xowB1; scalar: xowA -> xowB2;
  gpsimd: xo8a.  Own half split 3 ways so the z chunks and the own-half
  reduces unblock as early as the queues allow.
- Other half ships as fp8 natural-TILED 128-token blocks; its column sum
  runs on the PE as 16 tiny (block x ones) matmuls accumulating one
  PSUM [128,1] column -- the scalar engine only runs the 4 exps plus its
  share of the output scales.
- All 16 output tiles are single K=65 matmuls (pT row 64 = e^{-c} from the
  biased exp; vaug row 64 = [vtail | NTAIL]).  Opening split-K
  accumulations in a shared PSUM bank corrupts sibling slots, so no
  split-K: tiles simply wait for the tail vector, which the queue split
  makes early enough.
- 3 tiles per PSUM bank => one strided [128,3] reciprocal per bank.
- Scales (PSUM->SBUF bf16, ~0.35-0.45us each) are the drain wall: split
  DVE/ACT round-robin; stores spread over all three queues.
"""

import numpy as np

import concourse.bass as bass
import concourse.bacc as bacc
import concourse.tile as tile
from concourse import mybir
from concourse.bass_utils import run_bass_kernel_spmd

F32 = mybir.dt.float32
F16 = mybir.dt.float16
BF16 = mybir.dt.bfloat16
FP8 = mybir.dt.float8e4
AF = mybir.ActivationFunctionType
AX = mybir.AxisListType
ALU = mybir.AluOpType

B, S, E, KD = 4, 4096, 128, 64
HALF = S // 2            # tokens handled per core
NCORES = 8
CHUNK = 512              # tokens per z-matmul / exp
NCHUNK = HALF // CHUNK
TSUB = 128               # tokens per output tile
NTILE = HALF // TSUB     # 16
CSHIFT = 20.0            # fixed softmax shift
NTAIL = float(S - KD)    # 4032 all-zero score columns
W = E + 1                # 129: num cols + den col per tile
NBANK = 6                # 3 tiles per PSUM bank (last bank holds 1)

# wpk columns: [x64T(64) | WkT(64) | Wq(128, rows 0:64) | tri(64, rows 0:64)]
X64_OFF, WK_OFF, WV_OFF, WQ_OFF, TRI_OFF = 0, KD, 2 * KD, 2 * KD + E, 2 * KD + 2 * E
WPK_COLS = 2 * KD + 2 * E + KD  # 448


def _build_nc() -> bass.Bass:
    nc = bacc.Bacc("TRN2", target_bir_lowering=False, debug=False)

    wpk = nc.dram_tensor("wpk", [E, WPK_COLS], F16, kind="ExternalInput").ap()
    xowa = nc.dram_tensor("xowa", [E, 1024], F16, kind="ExternalInput").ap()
    xowb1 = nc.dram_tensor("xowb1", [E, 512], F16, kind="ExternalInput").ap()
    xowb2 = nc.dram_tensor("xowb2", [E, 512], F16, kind="ExternalInput").ap()
    xo8a = nc.dram_tensor("xo8a", [E, 1024], FP8, kind="ExternalInput").ap()
    xo8b = nc.dram_tensor("xo8b", [E, 1024], FP8, kind="ExternalInput").ap()
    outs = [
        nc.dram_tensor(f"o{g}", [TSUB, 4, E], BF16, kind="ExternalOutput").ap()
        for g in range(4)
    ]

    with tile.TileContext(nc) as tc:
        with (
            tc.tile_pool(name="singles", bufs=1) as singles,
            tc.tile_pool(name="zps", bufs=2, space="PSUM") as zps,
            tc.tile_pool(name="misc_ps", bufs=1, space="PSUM") as misc_ps,
            tc.tile_pool(name="oa_ps", bufs=4, space="PSUM") as oa_ps,
            tc.tile_pool(name="recs", bufs=3) as recs,
            tc.tile_pool(name="obs", bufs=3) as obs,
        ):
            # ---- DMA in (queue order == issue order per engine) ----
            wpk_sb = singles.tile([E, WPK_COLS], F16)
            nc.sync.dma_start(wpk_sb[:], wpk)
            xow_sb = singles.tile([E, HALF], F16)
            xo8_sb = singles.tile([E, HALF], FP8)
            nc.sync.dma_start(xo8_sb[:, 1024:2048], xo8b)
            nc.sync.dma_start(xow_sb[:, 1024:1536], xowb1)
            nc.scalar.dma_start(xow_sb[:, 0:1024], xowa)
            nc.scalar.dma_start(xow_sb[:, 1536:2048], xowb2)
            nc.gpsimd.dma_start(xo8_sb[:, 0:1024], xo8a)

            x64T_sb = wpk_sb[:, X64_OFF : X64_OFF + KD]
            wkT_sb = wpk_sb[:, WK_OFF : WK_OFF + KD]
            wvT_sb = wpk_sb[:, WV_OFF : WV_OFF + E]
            wq_sb = wpk_sb[0:KD, WQ_OFF : WQ_OFF + E]
            tri_sb = wpk_sb[0:KD, TRI_OFF : TRI_OFF + KD]

            # ---- constants (gpsimd is otherwise idle early) ----
            wzaug_sb = singles.tile([E, KD + 1], F16)
            nc.gpsimd.memset(wzaug_sb[:, KD : KD + 1], 0.0)
            vaug_sb = singles.tile([KD + 1, W], BF16)
            nc.gpsimd.memset(vaug_sb[0:KD, E : E + 1], 1.0)
            nc.gpsimd.memset(vaug_sb[KD : KD + 1, E : E + 1], NTAIL)
            nbias_sb = singles.tile([KD + 1, 1], F32)
            nc.gpsimd.memset(nbias_sb[:], -CSHIFT)
            ones8_sb = singles.tile([E, 1], FP8)
            nc.gpsimd.memset(ones8_sb[:], 1.0)

            # ---- preamble: build Wz (score weights) and v64 ----
            kT_ps = zps.tile([KD, KD], F32, tag="z", name="kT_ps")
            nc.tensor.matmul(kT_ps[:], wkT_sb, x64T_sb, start=True, stop=True)
            kmT_sb = singles.tile([KD, KD], F16)
            nc.vector.tensor_mul(kmT_sb[:], kT_ps[:], tri_sb)
            wzT_ps = zps.tile([E, KD], F32, tag="z", name="wzT_ps")
            nc.tensor.matmul(wzT_ps[:], wq_sb, kmT_sb[:], start=True, stop=True)
            nc.vector.tensor_copy(wzaug_sb[:, 0:KD], wzT_ps[:])
            v64_ps = zps.tile([KD, E], F32, tag="z", name="v64_ps")
            nc.tensor.matmul(v64_ps[:], x64T_sb, wvT_sb, start=True, stop=True)
            nc.scalar.activation(vaug_sb[0:KD, 0:E], v64_ps[:], AF.Copy)

            x64s_sb = singles.tile([E, 1], F32)
            nc.vector.reduce_sum(out=x64s_sb[:], in_=x64T_sb, axis=AX.X)

            # ---- z chunks + exps; fp8 PE-sums woven into PE gaps ----
            pT_sb = singles.tile([KD + 1, HALF], BF16)
            osum_ps = misc_ps.tile([E, 1], F32, name="osum_ps")

            def zexp(c):
                cs = slice(c * CHUNK, (c + 1) * CHUNK)
                z_ps = zps.tile([KD + 1, CHUNK], F32, tag="z", name=f"z{c}_ps")
                nc.tensor.matmul(
                    z_ps[:], wzaug_sb[:], xow_sb[:, cs], start=True, stop=True
                )
                nc.scalar.activation(
                    pT_sb[0 : KD + 1, cs], z_ps[:], AF.Exp, bias=nbias_sb[:]
                )

            def osums(half, first, last):
                for j in range(8):
                    js = slice(half * 1024 + j * TSUB, half * 1024 + (j + 1) * TSUB)
                    nc.tensor.matmul(
                        osum_ps[:], xo8_sb[:, js], ones8_sb[:],
                        start=(first and j == 0), stop=(last and j == 7),
                    )

            zexp(0)
            zexp(1)
            osums(0, True, False)    # gpsimd-queue half
            osums(1, False, True)    # sync-queue half
            zexp(2)
            zexp(3)

            # ---- own-half sums and the tail vector ----
            rdA_sb = singles.tile([E, 1], F32)
            nc.vector.reduce_sum(out=rdA_sb[:], in_=xow_sb[:, 0:1024], axis=AX.X)
            rdB_sb = singles.tile([E, 1], F32)
            nc.vector.reduce_sum(out=rdB_sb[:], in_=xow_sb[:, 1536:2048], axis=AX.X)
            rdC_sb = singles.tile([E, 1], F32)
            nc.vector.reduce_sum(out=rdC_sb[:], in_=xow_sb[:, 1024:1536], axis=AX.X)
            u1_sb = singles.tile([E, 1], F32)
            nc.vector.scalar_tensor_tensor(
                u1_sb[:], rdA_sb[:], rdB_sb[:], osum_ps[:], ALU.add, ALU.add
            )
            u2_sb = singles.tile([E, 1], F32)
            nc.vector.tensor_sub(u2_sb[:], rdC_sb[:], x64s_sb[:])
            tailh_sb = singles.tile([E, 1], F16)
            nc.vector.tensor_add(tailh_sb[:], u1_sb[:], u2_sb[:])

            vtail_ps = misc_ps.tile([1, E], F32, name="vtail_ps")
            nc.tensor.matmul(vtail_ps[:], tailh_sb[:], wvT_sb, start=True, stop=True)
            nc.vector.tensor_copy(vaug_sb[KD : KD + 1, 0:E], vtail_ps[:])

            # ---- output tiles: 3 per PSUM bank, K=65 single matmuls ----
            oa_banks = []

            def slot_ap(t):
                bank = oa_banks[t // 3]
                j = t % 3
                return bank[:, j * W : j * W + W]

            for t in range(NTILE):
                if t % 3 == 0:
                    oa_banks.append(
                        oa_ps.tile([TSUB, 3 * W], F32, tag="oa", name=f"oa{t // 3}")
                    )
                ts = slice(t * TSUB, (t + 1) * TSUB)
                nc.tensor.matmul(
                    slot_ap(t), pT_sb[0 : KD + 1, ts], vaug_sb[:],
                    start=True, stop=True,
                )

            # ---- normalize + store ----
            rec_tiles = []
            for b in range(NBANK):
                n = 3 if b < 5 else 1
                rec = recs.tile([TSUB, 3], F32, tag="rec", name=f"rec{b}")
                rec_tiles.append(rec)
                nc.vector.reciprocal(rec[:, 0:n], oa_banks[b][:, E :: W][:, 0:n])

            ob_tiles = []
            for t in range(NTILE):
                g, gj = divmod(t, 4)
                if gj == 0:
                    ob_tiles.append(
                        obs.tile([TSUB, 4, E], BF16, tag="ob", name=f"ob{g}")
                    )
                ob = ob_tiles[g]
                oa = slot_ap(t)
                rec = rec_tiles[t // 3][:, t % 3 : t % 3 + 1]
                if t % 8 in (0, 2, 4, 5, 6):
                    nc.vector.tensor_scalar_mul(ob[:, gj, :], oa[:, 0:E], rec)
                else:
                    nc.scalar.activation(ob[:, gj, :], oa[:, 0:E], AF.Copy, scale=rec)
                if gj == 3:
                    eng = (nc.scalar, nc.sync, nc.gpsimd, nc.scalar)[g]
                    eng.dma_start(outs[g], ob[:])

    nc.compile()
    return nc


_NC_CACHE = None


def _get_nc() -> bass.Bass:
    global _NC_CACHE
    if _NC_CACHE is None:
        _NC_CACHE = _build_nc()
    return _NC_CACHE


def _make_in_maps(x, Wk, Wq, Wv):
    tri = (np.arange(KD)[:, None] >= np.arange(KD)[None, :]).astype(np.float16)
    wq_pad = np.zeros((E, E), np.float16)
    wq_pad[:KD] = Wq.astype(np.float16)
    tri_pad = np.zeros((E, KD), np.float16)
    tri_pad[:KD] = tri
    x16 = x.astype(np.float16)
    fp8_np = mybir.dt.np(FP8)
    in_maps = []
    for c in range(NCORES):
        b, h = divmod(c, 2)
        xb_ = x16[b]
        wpk = np.concatenate(
            [
                xb_[:KD].T,
                Wk.T.astype(np.float16),
                Wv.T.astype(np.float16),
                wq_pad,
                tri_pad,
            ],
            axis=1,
        )
        own = xb_[h * HALF : (h + 1) * HALF].T  # [E, 2048]
        other = xb_[(1 - h) * HALF : (2 - h) * HALF]  # [2048, E] natural
        # natural-tiled fp8: block j holds tokens j*128..j*128+127 on the
        # partition axis: pack[p, j*128+e] = other[j*128+p, e]
        ot = other.astype(fp8_np).reshape(16, TSUB, E).transpose(1, 0, 2)
        ot = np.ascontiguousarray(ot).reshape(E, HALF)
        in_maps.append(
            {
                "wpk": np.ascontiguousarray(wpk),
                "xowa": np.ascontiguousarray(own[:, 0:1024]),
                "xowb1": np.ascontiguousarray(own[:, 1024:1536]),
                "xowb2": np.ascontiguousarray(own[:, 1536:2048]),
                "xo8a": np.ascontiguousarray(ot[:, 0:1024]),
                "xo8b": np.ascontiguousarray(ot[:, 1024:2048]),
            }
        )
    return in_maps


def _gather(results):
    out = np.empty((B, S, E), np.float32)
    for c, r in enumerate(results):
        b, h = divmod(c, 2)
        # per-group device layout [p, t, v], token = (4g + t)*128 + p
        dev = np.concatenate(
            [np.asarray(r[f"o{g}"], dtype=np.float32) for g in range(4)], axis=1
        )
        out[b, h * HALF : (h + 1) * HALF] = dev.transpose(1, 0, 2).reshape(HALF, E)
    return out


def _run(x, Wk, Wq, Wv, **spmd_kwargs):
    nc = _get_nc()
    res = run_bass_kernel_spmd(
        nc,
        _make_in_maps(x, Wk, Wq, Wv),
        core_ids=list(range(NCORES)),
        **spmd_kwargs,
    )
    return _gather(res.results), res


def kernel(x, Wk, Wq, Wv):
    x = np.ascontiguousarray(np.asarray(x), dtype=np.float32)
    Wk = np.ascontiguousarray(np.asarray(Wk), dtype=np.float32)
    Wq = np.ascontiguousarray(np.asarray(Wq), dtype=np.float32)
    Wv = np.ascontiguousarray(np.asarray(Wv), dtype=np.float32)
    out, _ = _run(x, Wk, Wq, Wv)
    return out
